# revision 62
# baseline (speedup 1.0000x reference)
"""GQA forward kernel for Trainium2, 8-core tensor-parallel (group-aligned).

Problem: B=2, T=2048, D=2048, 32 Q heads / 8 KV heads, head_dim 64, causal.

Sharding: core c = (batch b = c//4, kv-head pair j = c%4).  Each core owns
kv heads {2j, 2j+1} and their 8 q heads for ONE batch.  Each core reads only
its batch's x^T (fp16) and emits a row-parallel partial of the output
projection (fp16); the host sums 4 partials per batch (+ bo).

All matmuls in fp16 with fp32 PSUM accumulation (tolerance is 2e-2; fp16
keeps rel err ~1e-3 and always hits 1.0 cycles/row on the PE cost model).

Per-core dataflow:
  P1 (proj, 4 chunks of 512 tokens): lhsT = wqkv sub [C/16-slices, 128] fp16,
    rhs = x^T chunk -> 6 sub-blocks of 128: subs 0-3 = Q pairs [g0hi | g1hi],
    sub 4 = K2 = [K_g0 | K_g1]^T, sub 5 = V2^T (transposed to natural via PE
    identity-matmul transposes).  v2 layout [kv, 130] = [V_g0 | 1 | V_g1 | 1].
  P2 attention per (pair i, q-chunk of 512): scores transposed,
    S^T[kv, q] for both heads of the pair in one PSUM tile [128,2,512]
    (head g0hi contracts K2[0:64], g1hi contracts K2[64:128]).
    expS = ACT Exp(S/8) -> fp16 SBUF (all kv tiles of the chunk kept live);
    causal via column-sliced matmuls + one triangle mask multiply on
    diagonal tiles.
    AV is flipped: per [128q, 65] output region, lhsT = expS tile (stationary)
    and rhs = [V_h | ones] streams 65 columns - half the engine time of
    streaming query columns.  Each region is one sequential start->stop
    accumulation in its own PSUM bank (interleaved groups within a bank
    break on real hardware).  den lands in column 64, so normalize is a
    per-partition scalar multiply; a PE transpose restores attn^T.
  P3 out-proj: py[t,e] = sum_ks attn^T[128ks, t] @ wo[128ks, e], psum ->
    fp16 staging (DVE) -> one DMA per 128-token row block, deferred a few
    fillers so its wait never blocks the SP queue head.
  Scheduling: attention chunk qc runs right after proj chunk qc; proj chunk
  qc+1 and out-proj chunk qc-1 are emitted one matmul at a time between
  attention tiles as PE filler while ACT works through the exps.
"""

import os

import numpy as np

import concourse.mybir as mybir
import concourse.tile as tile
from concourse import bacc
from concourse import bass_utils

P = 128
B = 2
T = 2048
C = 2048
HD = 64
QH = 32
KVH = 8
NCORES = 8
TCH = 512   # token chunk for projection phase
QCH = 512   # q chunk for attention phase
KT = C // P  # 16 contraction tiles
NSUB = 6     # 4 Q pairs + K2 + V2
f32 = mybir.dt.float32
fp16 = mybir.dt.float16

_CACHE = {}


def _build():
    nc = bacc.Bacc("TRN2", target_bir_lowering=False, debug=False, num_devices=NCORES)

    xt = nc.dram_tensor("xt", [C, T], fp16, kind="ExternalInput")
    # sub-major, pre-rearranged on host: [sub, p, ko, m] so a per-sub load is
    # one 4KB-descriptor DMA
    wqkv = nc.dram_tensor("wqkv", [NSUB, P, KT, P], fp16, kind="ExternalInput")
    wo = nc.dram_tensor("wo", [4 * P, C], fp16, kind="ExternalInput")
    bqkv = nc.dram_tensor("bqkv", [P, NSUB], f32, kind="ExternalInput")
    maskd = nc.dram_tensor("mask", [P, P], fp16, kind="ExternalInput")
    identd = nc.dram_tensor("ident", [P, P], fp16, kind="ExternalInput")
    y = nc.dram_tensor("y", [T, C], fp16, kind="ExternalOutput")

    wo3 = wo.ap().rearrange("(ko p) m -> p ko m", p=P)
    xb = xt.ap().rearrange("(ko p) t -> p ko t", p=P)

    Exp = mybir.ActivationFunctionType.Exp
    mult = mybir.AluOpType.mult
    add = mybir.AluOpType.add

    with tile.TileContext(nc) as tc:
        with (
            tc.tile_pool(name="const", bufs=1) as cpool,
            tc.tile_pool(name="x", bufs=2) as xpool,
            tc.tile_pool(name="res", bufs=1) as apool,
            tc.tile_pool(name="vt", bufs=2) as vtpool,
            tc.tile_pool(name="expS", bufs=16) as wpool,
            tc.tile_pool(name="den", bufs=2) as dpool,
            tc.tile_pool(name="y", bufs=2) as ypool,
            tc.tile_pool(name="psA", bufs=2, space="PSUM") as psumA,
            tc.tile_pool(name="psB", bufs=2, space="PSUM") as psumB,
            tc.tile_pool(name="psC", bufs=2, space="PSUM") as psumC,
        ):
            # ---- constants / weights (resident) ----
            # startup-critical DMA order: wqkv sub0, x chunk 0 halves (the
            # first 16 proj matmuls need only these), then the rest.
            wqkv_subs = []
            for _s in range(NSUB):
                w_s = cpool.tile([P, KT, P], fp16, tag=f"w{_s}", name="w_s")
                wqkv_subs.append(w_s)

            def xch_alloc(tch):
                # two tiles so matmuls on the first 8 K-slices need not wait
                # for the second half's DMA
                xlo = xpool.tile([P, KT // 2, TCH], fp16, tag="xlo", name="xlo")
                xhi = xpool.tile([P, KT // 2, TCH], fp16, tag="xhi", name="xhi")
                tsl = slice(tch * TCH, (tch + 1) * TCH)
                nc.sync.dma_start(xlo[:], xb[:, 0 : KT // 2, tsl])
                nc.sync.dma_start(xhi[:], xb[:, KT // 2 :, tsl])
                return (xlo, xhi)

            # startup-critical order: sub0 weights, x chunk 0 halves, then
            # remaining subs one DMA each (4KB descriptors)
            nc.sync.dma_start(wqkv_subs[0][:], wqkv.ap()[0])
            xch0 = xch_alloc(0)
            for _s in range(1, NSUB):
                nc.sync.dma_start(wqkv_subs[_s][:], wqkv.ap()[_s])
            bqkv_sb = cpool.tile([P, NSUB], f32)
            nc.sync.dma_start(bqkv_sb[:], bqkv.ap())
            mask_sb = cpool.tile([P, P], fp16)
            nc.sync.dma_start(mask_sb[:], maskd.ap())
            ident_sb = cpool.tile([P, P], fp16)
            nc.sync.dma_start(ident_sb[:], identd.ap())
            wo_sb = cpool.tile([P, 4, C], fp16)

            q_sb = apool.tile([P, 4, T], fp16, tag="q")
            k2_sb = apool.tile([P, T], fp16, tag="k2")
            # v2 cols: [V_g0 (0:64) | ones (64) | V_g1 (65:129) | ones (129)]
            v2_sb = apool.tile([P, KT, 130], fp16, tag="v2")
            attn_sb = apool.tile([P, 4, T], fp16, tag="attn")
            nc.gpsimd.memset(v2_sb[:, :, 64:65], 1.0)
            nc.gpsimd.memset(v2_sb[:, :, 129:130], 1.0)

            # ---- filler queues: closures each emitting ~one PE matmul.
            # proj fillers have a deadline (their attention chunk) and pop
            # first; p3 fillers drain opportunistically.
            fillers_proj = []
            fillers_p3 = []

            def pop_filler(k=1):
                for _ in range(k):
                    if fillers_proj:
                        fillers_proj.pop(0)()
                    elif fillers_p3:
                        fillers_p3.pop(0)()

            def make_p3_fillers(qc):
                """Out-proj for token range [q0, q0+qch): one 128-token
                row-block per ts, 4 col-chunks each a 4-matmul psum
                accumulation + DVE copy; one DMA per row-block."""
                q0, qch = qc
                out = []
                prev_dma = None
                for ts in range(q0 // P, (q0 + qch) // P):
                    state = {}
                    mms = []
                    for ec in range(C // QCH):
                        for ks in range(4):
                            def mm(ts=ts, ec=ec, ks=ks, state=state):
                                if ks == 0 and ec == 0:
                                    state["y"] = ypool.tile(
                                        [P, C], fp16, tag="ysb", name="ysb"
                                    )
                                if ks == 0:
                                    state["py"] = psumC.tile(
                                        [P, QCH], f32, tag="pp", name="py"
                                    )
                                nc.tensor.matmul(
                                    state["py"][:],
                                    attn_sb[:, ks, ts * P : (ts + 1) * P],
                                    wo_sb[:, ks, ec * QCH : (ec + 1) * QCH],
                                    start=(ks == 0),
                                    stop=(ks == 3),
                                    skip_group_check=True,
                                )
                                if ks == 3:
                                    nc.vector.tensor_copy(
                                        state["y"][:, ec * QCH : (ec + 1) * QCH],
                                        state["py"][:],
                                    )
                            mm.pe_ns = 213
                            mms.append(mm)

                    def ydma(ts=ts, state=state):
                        nc.sync.dma_start(
                            y.ap()[ts * P : (ts + 1) * P, :], state["y"][:]
                        )

                    ydma.pe_ns = 0

                    # defer each row-block's output DMA a few fillers past its
                    # last staging copy so its wait is satisfied when the SP
                    # queue reaches it
                    out.extend(mms[:4])
                    if prev_dma is not None:
                        out.append(prev_dma)
                    out.extend(mms[4:])
                    prev_dma = ydma
                out.append(prev_dma)
                return out

            # ---- P1: one projection chunk, as a list of filler closures ----
            def proj_fillers(tch, xch):
                tsl = slice(tch * TCH, (tch + 1) * TCH)
                out = []
                for sub in range(NSUB):
                    state = {}
                    for k in range(KT):
                        def mm(sub=sub, k=k, state=state):
                            if k == 0:
                                state["pp"] = psumC.tile(
                                    [P, TCH], f32, tag="pp", name="pp"
                                )
                            nc.tensor.matmul(
                                state["pp"][:],
                                wqkv_subs[sub][:, k, :],
                                xch[k // (KT // 2)][:, k % (KT // 2), :],
                                start=(k == 0),
                                stop=(k == KT - 1),
                                skip_group_check=True,
                            )
                            if k == KT - 1:
                                bias = bqkv_sb[:, sub : sub + 1].to_broadcast(
                                    (P, TCH)
                                )
                                if sub < 4:
                                    nc.vector.tensor_tensor(
                                        q_sb[:, sub, tsl], state["pp"][:], bias, add
                                    )
                                elif sub == 4:
                                    nc.vector.tensor_tensor(
                                        k2_sb[:, tsl], state["pp"][:], bias, add
                                    )
                                else:
                                    state["vt"] = vtpool.tile(
                                        [P, TCH], fp16, tag="vt", name="vt"
                                    )
                                    nc.vector.tensor_tensor(
                                        state["vt"][:], state["pp"][:], bias, add
                                    )
                        mm.pe_ns = 213
                        out.append(mm)
                    if sub == 5:
                        for ts in range(TCH // P):
                            def tpose(ts=ts, tch=tch, state=state):
                                tidx = tch * (TCH // P) + ts
                                pt = psumC.tile([P, P], fp16, tag="pp", name="pt")
                                nc.tensor.transpose(
                                    pt[:],
                                    state["vt"][:, ts * P : (ts + 1) * P],
                                    ident_sb[:],
                                )
                                nc.vector.tensor_copy(v2_sb[:, tidx, 0:64], pt[:, 0:64])
                                nc.vector.tensor_copy(
                                    v2_sb[:, tidx, 65:129], pt[:, 64:128]
                                )
                            tpose.pe_ns = 53
                            out.append(tpose)
                return out

            def proj_chunk(tch, xch):
                for fn in proj_fillers(tch, xch):
                    fn()

            # ---- P2: attention for one (pair, q-chunk) ----
            # AV is "flipped": expS tiles are the PE stationary operand and V
            # streams through (65 cols instead of up to 512), halving AV
            # engine time.  Output pq[q, d] has the denominator at column 64
            # of each region, so normalize is a per-partition scalar multiply;
            # a PE transpose restores attn^T for the output projection.
            def attn_pair(q0, qch, pair):
                nq = qch // P
                nfull = q0 // P
                ntiles = nfull + nq
                exps = []
                for i in range(ntiles):
                    lo = 0 if i < nfull else (i - nfull) * P
                    nsl = slice(lo, qch)
                    qsl = slice(q0 + lo, q0 + qch)
                    ksl = slice(i * P, (i + 1) * P)
                    ps_s = psumA.tile([P, 2, QCH], f32, tag="ps", name="ps_s")
                    nc.tensor.matmul(
                        ps_s[:, 0, nsl],
                        k2_sb[0:64, ksl],
                        q_sb[0:64, pair, qsl],
                        start=True,
                        stop=True,
                        skip_group_check=True,
                    )
                    nc.tensor.matmul(
                        ps_s[:, 1, nsl],
                        k2_sb[64:128, ksl],
                        q_sb[64:128, pair, qsl],
                        start=True,
                        stop=True,
                        skip_group_check=True,
                    )
                    expS = wpool.tile([P, 2, QCH], fp16, tag="expS")
                    nc.scalar.activation(expS[:, :, nsl], ps_s[:, :, nsl], Exp, scale=0.125)
                    if i >= nfull:
                        j = i - nfull
                        jsl = slice(j * P, (j + 1) * P)
                        nc.vector.tensor_tensor(
                            expS[:, :, jsl],
                            expS[:, :, jsl],
                            mask_sb[:, None, :].to_broadcast((P, 2, P)),
                            mult,
                        )
                    exps.append(expS)
                    pop_filler(2)
                # AV per [128q, 65] region: one sequential accumulation group
                # per PSUM bank; den lands in column 64, so normalize is a
                # per-partition scalar multiply; a PE transpose restores
                # attn^T for the output projection.
                qsl = slice(q0, q0 + qch)
                tp = psumC.tile([P, 4, P], fp16, tag="pp", name="tp")
                for h in range(2):
                    rec = dpool.tile([P, 4, 1], f32, tag="rec")
                    attn_n = dpool.tile([P, 4, HD], fp16, tag="attn_n")
                    for jj in range(nq):
                        pq = psumB.tile([P, 65], f32, tag="pq", name="pq")
                        last = nfull + jj
                        for i in range(last + 1):
                            nc.tensor.matmul(
                                pq[:],
                                exps[i][:, h, jj * P : (jj + 1) * P],
                                v2_sb[:, i, h * 65 : h * 65 + 65],
                                start=(i == 0),
                                stop=(i == last),
                                skip_group_check=True,
                            )
                        nc.vector.reciprocal(rec[:, jj, :], pq[:, 64:65])
                        nc.vector.tensor_tensor(
                            attn_n[:, jj, :],
                            pq[:, 0:64],
                            rec[:, jj, :].to_broadcast((P, HD)),
                            mult,
                        )
                        nc.tensor.transpose(
                            tp[h * HD : (h + 1) * HD, jj, :],
                            attn_n[:, jj, :],
                            ident_sb[:],
                        )
                        pop_filler(1)
                nc.vector.tensor_copy(attn_sb[:, pair, qsl], tp[:, 0:nq, :])

            # ---- main schedule ----
            # proj(0) runs as a block; proj(qc+1) and out-proj(qc-1) pop as
            # fillers between attention tiles of chunk qc, keeping the PE fed
            # while ACT works through the exps.  The last token chunk's
            # attention is split in two so most of its out-proj still has
            # attention to overlap with.
            proj_chunk(0, xch0)
            nc.sync.dma_start(wo_sb[:], wo3)
            sched = [(0, TCH), (TCH, TCH), (2 * TCH, TCH), (3 * TCH, TCH)]
            for s, (q0, qch) in enumerate(sched):
                if s < 3:
                    fillers_proj.extend(proj_fillers(s + 1, xch_alloc(s + 1)))
                for pair in range(4):
                    attn_pair(q0, qch, pair)
                    pop_filler(2)
                # the next chunk's attention needs its projections complete
                while fillers_proj:
                    pop_filler(1)
                fillers_p3.extend(make_p3_fillers((q0, qch)))
            while fillers_p3:
                pop_filler(1)

    nc.compile()
    return nc


def _prep_inputs(x, Wq, bq, Wk, bk, Wv, bv, Wo, bo):
    x = np.ascontiguousarray(np.asarray(x, dtype=np.float32))
    Wq = np.asarray(Wq, dtype=np.float32)
    Wk = np.asarray(Wk, dtype=np.float32)
    Wv = np.asarray(Wv, dtype=np.float32)
    Wo = np.asarray(Wo, dtype=np.float32)
    bq = np.asarray(bq, dtype=np.float32)
    bk = np.asarray(bk, dtype=np.float32)
    bv = np.asarray(bv, dtype=np.float32)

    xts = [np.ascontiguousarray(x[b].T).astype(np.float16) for b in range(B)]
    # mask[kj, qi] = 1 iff kj <= qi  (upper triangular incl. diag)
    mask = np.triu(np.ones((P, P), dtype=np.float16)).copy()
    ident = np.eye(P, dtype=np.float16)
    in_maps = []
    for c in range(NCORES):
        b, j = divmod(c, 4)
        # q heads: g0 = 8j..8j+3 (kv head 2j), g1 = 8j+4..8j+7 (kv 2j+1)
        qcols, wocols, bqc = [], [], []
        for i in range(4):
            for h in (8 * j + i, 8 * j + 4 + i):
                qcols.append(Wq[:, h * HD : (h + 1) * HD])
                wocols.append(Wo[h * HD : (h + 1) * HD, :])
                bqc.append(bq[h * HD : (h + 1) * HD])
        ks = slice(2 * j * HD, (2 * j + 2) * HD)
        wqkv_c = np.concatenate(qcols + [Wk[:, ks], Wv[:, ks]], axis=1)
        # [C, 768] -> [sub, p, ko, m] (sub-major, 4KB contiguous per (sub, p))
        wqkv_r = wqkv_c.reshape(KT, P, NSUB, P).transpose(2, 1, 0, 3)
        bqkv_c = np.stack(
            [np.concatenate(bqc[2 * i : 2 * i + 2]) for i in range(4)]
            + [bk[ks], bv[ks]],
            axis=1,
        )
        in_maps.append(
            {
                "xt": xts[b],
                "wqkv": np.ascontiguousarray(wqkv_r).astype(np.float16),
                "wo": np.ascontiguousarray(np.concatenate(wocols, axis=0)).astype(
                    np.float16
                ),
                "bqkv": np.ascontiguousarray(bqkv_c),
                "mask": mask,
                "ident": ident,
            }
        )
    return in_maps


def kernel(x, Wq, bq, Wk, bk, Wv, bv, Wo, bo, _trace=False):
    # NTFF tracing is unavailable through this axon client; make sure a
    # stray BASS_TRACE=1 in the environment cannot divert the run path.
    if not _trace:
        os.environ["BASS_NEVER_TRACE"] = "1"
    if "nc" not in _CACHE:
        _CACHE["nc"] = _build()
    nc = _CACHE["nc"]
    in_maps = _prep_inputs(x, Wq, bq, Wk, bk, Wv, bv, Wo, bo)
    res = bass_utils.run_bass_kernel_spmd(
        nc, in_maps, core_ids=list(range(NCORES)), trace=_trace
    )
    bo = np.asarray(bo, dtype=np.float32)
    y = np.zeros((B, T, C), dtype=np.float32)
    for c in range(NCORES):
        y[c // 4] += res.results[c]["y"].astype(np.float32)
    y += bo
    if _trace:
        return y, res
    return y


# revision 67
# speedup vs baseline: 1.0013x; 1.0013x over previous
"""GQA forward kernel for Trainium2, 8-core tensor-parallel (group-aligned).

Problem: B=2, T=2048, D=2048, 32 Q heads / 8 KV heads, head_dim 64, causal.

Sharding: core c = (batch b = c//4, kv-head pair j = c%4).  Each core owns
kv heads {2j, 2j+1} and their 8 q heads for ONE batch.  Each core reads only
its batch's x^T (fp16) and emits a row-parallel partial of the output
projection (fp16); the host sums 4 partials per batch (+ bo).

All matmuls in fp16 with fp32 PSUM accumulation (tolerance is 2e-2; fp16
keeps rel err ~1e-3 and always hits 1.0 cycles/row on the PE cost model).

Per-core dataflow:
  P1 (proj, 4 chunks of 512 tokens): lhsT = wqkv sub [C/16-slices, 128] fp16,
    rhs = x^T chunk -> 6 sub-blocks of 128: subs 0-3 = Q pairs [g0hi | g1hi],
    sub 4 = K2 = [K_g0 | K_g1]^T, sub 5 = V2^T (transposed to natural via PE
    identity-matmul transposes).  v2 layout [kv, 130] = [V_g0 | 1 | V_g1 | 1].
  P2 attention per (pair i, q-chunk of 512): scores transposed,
    S^T[kv, q] for both heads of the pair in one PSUM tile [128,2,512]
    (head g0hi contracts K2[0:64], g1hi contracts K2[64:128]).
    expS = ACT Exp(S/8) -> fp16 SBUF (all kv tiles of the chunk kept live);
    causal via column-sliced matmuls + one triangle mask multiply on
    diagonal tiles.
    AV is flipped: per [128q, 65] output region, lhsT = expS tile (stationary)
    and rhs = [V_h | ones] streams 65 columns - half the engine time of
    streaming query columns.  Each region is one sequential start->stop
    accumulation in its own PSUM bank (interleaved groups within a bank
    break on real hardware).  den lands in column 64, so normalize is a
    per-partition scalar multiply; a PE transpose restores attn^T.
  P3 out-proj: py[t,e] = sum_ks attn^T[128ks, t] @ wo[128ks, e], psum ->
    fp16 staging (DVE) -> one DMA per 128-token row block, deferred a few
    fillers so its wait never blocks the SP queue head.
  Scheduling: attention chunk qc runs right after proj chunk qc; proj chunk
  qc+1 and out-proj chunk qc-1 are emitted one matmul at a time between
  attention tiles as PE filler while ACT works through the exps.
"""

import os

import numpy as np

import concourse.mybir as mybir
import concourse.tile as tile
from concourse import bacc
from concourse import bass_utils

P = 128
B = 2
T = 2048
C = 2048
HD = 64
QH = 32
KVH = 8
NCORES = 8
TCH = 512   # token chunk for projection phase
QCH = 512   # q chunk for attention phase
KT = C // P  # 16 contraction tiles
NSUB = 6     # 4 Q pairs + K2 + V2
f32 = mybir.dt.float32
fp16 = mybir.dt.float16

_CACHE = {}


def _build():
    nc = bacc.Bacc("TRN2", target_bir_lowering=False, debug=False, num_devices=NCORES)

    xt = nc.dram_tensor("xt", [C, T], fp16, kind="ExternalInput")
    # sub-major, pre-rearranged on host: [sub, p, ko, m] so a per-sub load is
    # one 4KB-descriptor DMA
    wqkv = nc.dram_tensor("wqkv", [NSUB, P, KT, P], fp16, kind="ExternalInput")
    wo = nc.dram_tensor("wo", [4 * P, C], fp16, kind="ExternalInput")
    bqkv = nc.dram_tensor("bqkv", [P, NSUB], f32, kind="ExternalInput")
    maskd = nc.dram_tensor("mask", [P, P], fp16, kind="ExternalInput")
    identd = nc.dram_tensor("ident", [P, P], fp16, kind="ExternalInput")
    y = nc.dram_tensor("y", [T, C], fp16, kind="ExternalOutput")

    wo3 = wo.ap().rearrange("(ko p) m -> p ko m", p=P)
    xb = xt.ap().rearrange("(ko p) t -> p ko t", p=P)

    Exp = mybir.ActivationFunctionType.Exp
    mult = mybir.AluOpType.mult
    add = mybir.AluOpType.add

    with tile.TileContext(nc) as tc:
        with (
            tc.tile_pool(name="const", bufs=1) as cpool,
            tc.tile_pool(name="x", bufs=2) as xpool,
            tc.tile_pool(name="res", bufs=1) as apool,
            tc.tile_pool(name="vt", bufs=2) as vtpool,
            tc.tile_pool(name="expS", bufs=16) as wpool,
            tc.tile_pool(name="den", bufs=2) as dpool,
            tc.tile_pool(name="y", bufs=2) as ypool,
            tc.tile_pool(name="psA", bufs=2, space="PSUM") as psumA,
            tc.tile_pool(name="psB", bufs=2, space="PSUM") as psumB,
            tc.tile_pool(name="psC", bufs=2, space="PSUM") as psumC,
        ):
            # ---- constants / weights (resident) ----
            # startup-critical DMA order: wqkv sub0, x chunk 0 halves (the
            # first 16 proj matmuls need only these), then the rest.
            wqkv_subs = []
            for _s in range(NSUB):
                w_s = cpool.tile([P, KT, P], fp16, tag=f"w{_s}", name="w_s")
                wqkv_subs.append(w_s)

            def xch_alloc(tch):
                # two tiles so matmuls on the first 8 K-slices need not wait
                # for the second half's DMA
                xlo = xpool.tile([P, KT // 2, TCH], fp16, tag="xlo", name="xlo")
                xhi = xpool.tile([P, KT // 2, TCH], fp16, tag="xhi", name="xhi")
                tsl = slice(tch * TCH, (tch + 1) * TCH)
                nc.sync.dma_start(xlo[:], xb[:, 0 : KT // 2, tsl])
                nc.sync.dma_start(xhi[:], xb[:, KT // 2 :, tsl])
                return (xlo, xhi)

            # startup-critical order: sub0 weights, x chunk 0 halves, then
            # remaining subs one DMA each (4KB descriptors)
            nc.sync.dma_start(wqkv_subs[0][:], wqkv.ap()[0])
            xch0 = xch_alloc(0)
            for _s in (1, 2, 3, 5, 4):
                nc.sync.dma_start(wqkv_subs[_s][:], wqkv.ap()[_s])
            bqkv_sb = cpool.tile([P, NSUB], f32)
            nc.sync.dma_start(bqkv_sb[:], bqkv.ap())
            mask_sb = cpool.tile([P, P], fp16)
            nc.sync.dma_start(mask_sb[:], maskd.ap())
            ident_sb = cpool.tile([P, P], fp16)
            nc.sync.dma_start(ident_sb[:], identd.ap())
            wo_sb = cpool.tile([P, 4, C], fp16)

            q_sb = apool.tile([P, 4, T], fp16, tag="q")
            k2_sb = apool.tile([P, T], fp16, tag="k2")
            # v2 cols: [V_g0 (0:64) | ones (64) | V_g1 (65:129) | ones (129)]
            v2_sb = apool.tile([P, KT, 130], fp16, tag="v2")
            attn_sb = apool.tile([P, 4, T], fp16, tag="attn")
            nc.gpsimd.memset(v2_sb[:, :, 64:65], 1.0)
            nc.gpsimd.memset(v2_sb[:, :, 129:130], 1.0)

            # ---- filler queues: closures each emitting ~one PE matmul.
            # proj fillers have a deadline (their attention chunk) and pop
            # first; p3 fillers drain opportunistically.
            fillers_proj = []
            fillers_p3 = []

            def pop_filler(k=1):
                for _ in range(k):
                    if fillers_proj:
                        fillers_proj.pop(0)()
                    elif fillers_p3:
                        fillers_p3.pop(0)()

            def make_p3_fillers(qc):
                """Out-proj for token range [q0, q0+qch): one 128-token
                row-block per ts, 4 col-chunks each a 4-matmul psum
                accumulation + DVE copy; one DMA per row-block."""
                q0, qch = qc
                out = []
                prev_dma = None
                for ts in range(q0 // P, (q0 + qch) // P):
                    state = {}
                    mms = []
                    for ec in range(C // QCH):
                        for ks in range(4):
                            def mm(ts=ts, ec=ec, ks=ks, state=state):
                                if ks == 0 and ec == 0:
                                    state["y"] = ypool.tile(
                                        [P, C], fp16, tag="ysb", name="ysb"
                                    )
                                if ks == 0:
                                    state["py"] = psumC.tile(
                                        [P, QCH], f32, tag="pp", name="py"
                                    )
                                nc.tensor.matmul(
                                    state["py"][:],
                                    attn_sb[:, ks, ts * P : (ts + 1) * P],
                                    wo_sb[:, ks, ec * QCH : (ec + 1) * QCH],
                                    start=(ks == 0),
                                    stop=(ks == 3),
                                    skip_group_check=True,
                                )
                                if ks == 3:
                                    nc.vector.tensor_copy(
                                        state["y"][:, ec * QCH : (ec + 1) * QCH],
                                        state["py"][:],
                                    )
                            mm.pe_ns = 213
                            mms.append(mm)

                    def ydma(ts=ts, state=state):
                        nc.sync.dma_start(
                            y.ap()[ts * P : (ts + 1) * P, :], state["y"][:]
                        )

                    ydma.pe_ns = 0

                    # defer each row-block's output DMA a few fillers past its
                    # last staging copy so its wait is satisfied when the SP
                    # queue reaches it
                    out.extend(mms[:4])
                    if prev_dma is not None:
                        out.append(prev_dma)
                    out.extend(mms[4:])
                    prev_dma = ydma
                out.append(prev_dma)
                return out

            # ---- P1: one projection chunk, as a list of filler closures ----
            def proj_fillers(tch, xch):
                tsl = slice(tch * TCH, (tch + 1) * TCH)
                out = []
                tposes = []
                for sub in (0, 1, 2, 3, 5, 4):
                    state = {}
                    for k in range(KT):
                        def mm(sub=sub, k=k, state=state):
                            if k == 0:
                                state["pp"] = psumC.tile(
                                    [P, TCH], f32, tag="pp", name="pp"
                                )
                            nc.tensor.matmul(
                                state["pp"][:],
                                wqkv_subs[sub][:, k, :],
                                xch[k // (KT // 2)][:, k % (KT // 2), :],
                                start=(k == 0),
                                stop=(k == KT - 1),
                                skip_group_check=True,
                            )
                            if k == KT - 1:
                                bias = bqkv_sb[:, sub : sub + 1].to_broadcast(
                                    (P, TCH)
                                )
                                if sub < 4:
                                    nc.vector.tensor_tensor(
                                        q_sb[:, sub, tsl], state["pp"][:], bias, add
                                    )
                                elif sub == 4:
                                    nc.vector.tensor_tensor(
                                        k2_sb[:, tsl], state["pp"][:], bias, add
                                    )
                                else:
                                    state["vt"] = vtpool.tile(
                                        [P, TCH], fp16, tag="vt", name="vt"
                                    )
                                    nc.vector.tensor_tensor(
                                        state["vt"][:], state["pp"][:], bias, add
                                    )
                        mm.pe_ns = 213
                        out.append(mm)
                    if sub == 5:
                        for ts in range(TCH // P):
                            def tpose(ts=ts, tch=tch, state=state):
                                tidx = tch * (TCH // P) + ts
                                pt = psumC.tile([P, P], fp16, tag="pp", name="pt")
                                nc.tensor.transpose(
                                    pt[:],
                                    state["vt"][:, ts * P : (ts + 1) * P],
                                    ident_sb[:],
                                )
                                nc.vector.tensor_copy(v2_sb[:, tidx, 0:64], pt[:, 0:64])
                                nc.vector.tensor_copy(
                                    v2_sb[:, tidx, 65:129], pt[:, 64:128]
                                )
                            tposes.append(tpose)
                out.extend(tposes)
                return out

            def proj_chunk(tch, xch):
                for fn in proj_fillers(tch, xch):
                    fn()

            # ---- P2: attention for one (pair, q-chunk) ----
            # AV is "flipped": expS tiles are the PE stationary operand and V
            # streams through (65 cols instead of up to 512), halving AV
            # engine time.  Output pq[q, d] has the denominator at column 64
            # of each region, so normalize is a per-partition scalar multiply;
            # a PE transpose restores attn^T for the output projection.
            def attn_pair(q0, qch, pair):
                nq = qch // P
                nfull = q0 // P
                ntiles = nfull + nq
                exps = []
                for i in range(ntiles):
                    lo = 0 if i < nfull else (i - nfull) * P
                    nsl = slice(lo, qch)
                    qsl = slice(q0 + lo, q0 + qch)
                    ksl = slice(i * P, (i + 1) * P)
                    ps_s = psumA.tile([P, 2, QCH], f32, tag="ps", name="ps_s")
                    nc.tensor.matmul(
                        ps_s[:, 0, nsl],
                        k2_sb[0:64, ksl],
                        q_sb[0:64, pair, qsl],
                        start=True,
                        stop=True,
                        skip_group_check=True,
                    )
                    nc.tensor.matmul(
                        ps_s[:, 1, nsl],
                        k2_sb[64:128, ksl],
                        q_sb[64:128, pair, qsl],
                        start=True,
                        stop=True,
                        skip_group_check=True,
                    )
                    expS = wpool.tile([P, 2, QCH], fp16, tag="expS")
                    nc.scalar.activation(expS[:, :, nsl], ps_s[:, :, nsl], Exp, scale=0.125)
                    if i >= nfull:
                        j = i - nfull
                        jsl = slice(j * P, (j + 1) * P)
                        nc.vector.tensor_tensor(
                            expS[:, :, jsl],
                            expS[:, :, jsl],
                            mask_sb[:, None, :].to_broadcast((P, 2, P)),
                            mult,
                        )
                    exps.append(expS)
                    pop_filler(2)
                # AV per [128q, 65] region: one sequential accumulation group
                # per PSUM bank; den lands in column 64, so normalize is a
                # per-partition scalar multiply; a PE transpose restores
                # attn^T for the output projection.
                qsl = slice(q0, q0 + qch)
                tp = psumC.tile([P, 4, P], fp16, tag="pp", name="tp")
                for h in range(2):
                    rec = dpool.tile([P, 4, 1], f32, tag="rec")
                    attn_n = dpool.tile([P, 4, HD], fp16, tag="attn_n")
                    for jj in range(nq):
                        pq = psumB.tile([P, 65], f32, tag="pq", name="pq")
                        last = nfull + jj
                        for i in range(last + 1):
                            nc.tensor.matmul(
                                pq[:],
                                exps[i][:, h, jj * P : (jj + 1) * P],
                                v2_sb[:, i, h * 65 : h * 65 + 65],
                                start=(i == 0),
                                stop=(i == last),
                                skip_group_check=True,
                            )
                        nc.vector.reciprocal(rec[:, jj, :], pq[:, 64:65])
                        nc.vector.tensor_tensor(
                            attn_n[:, jj, :],
                            pq[:, 0:64],
                            rec[:, jj, :].to_broadcast((P, HD)),
                            mult,
                        )
                        nc.tensor.transpose(
                            tp[h * HD : (h + 1) * HD, jj, :],
                            attn_n[:, jj, :],
                            ident_sb[:],
                        )
                        pop_filler(1)
                nc.vector.tensor_copy(attn_sb[:, pair, qsl], tp[:, 0:nq, :])

            # ---- main schedule ----
            # proj(0) runs as a block; proj(qc+1) and out-proj(qc-1) pop as
            # fillers between attention tiles of chunk qc, keeping the PE fed
            # while ACT works through the exps.  The last token chunk's
            # attention is split in two so most of its out-proj still has
            # attention to overlap with.
            proj_chunk(0, xch0)
            nc.sync.dma_start(wo_sb[:], wo3)
            sched = [(0, TCH), (TCH, TCH), (2 * TCH, TCH), (3 * TCH, TCH)]
            for s, (q0, qch) in enumerate(sched):
                if s < 3:
                    fillers_proj.extend(proj_fillers(s + 1, xch_alloc(s + 1)))
                for pair in range(4):
                    attn_pair(q0, qch, pair)
                    pop_filler(2)
                # the next chunk's attention needs its projections complete
                while fillers_proj:
                    pop_filler(1)
                fillers_p3.extend(make_p3_fillers((q0, qch)))
            while fillers_p3:
                pop_filler(1)

    nc.compile()
    return nc


def _prep_inputs(x, Wq, bq, Wk, bk, Wv, bv, Wo, bo):
    x = np.ascontiguousarray(np.asarray(x, dtype=np.float32))
    Wq = np.asarray(Wq, dtype=np.float32)
    Wk = np.asarray(Wk, dtype=np.float32)
    Wv = np.asarray(Wv, dtype=np.float32)
    Wo = np.asarray(Wo, dtype=np.float32)
    bq = np.asarray(bq, dtype=np.float32)
    bk = np.asarray(bk, dtype=np.float32)
    bv = np.asarray(bv, dtype=np.float32)

    xts = [np.ascontiguousarray(x[b].T).astype(np.float16) for b in range(B)]
    # mask[kj, qi] = 1 iff kj <= qi  (upper triangular incl. diag)
    mask = np.triu(np.ones((P, P), dtype=np.float16)).copy()
    ident = np.eye(P, dtype=np.float16)
    in_maps = []
    for c in range(NCORES):
        b, j = divmod(c, 4)
        # q heads: g0 = 8j..8j+3 (kv head 2j), g1 = 8j+4..8j+7 (kv 2j+1)
        qcols, wocols, bqc = [], [], []
        for i in range(4):
            for h in (8 * j + i, 8 * j + 4 + i):
                qcols.append(Wq[:, h * HD : (h + 1) * HD])
                wocols.append(Wo[h * HD : (h + 1) * HD, :])
                bqc.append(bq[h * HD : (h + 1) * HD])
        ks = slice(2 * j * HD, (2 * j + 2) * HD)
        wqkv_c = np.concatenate(qcols + [Wk[:, ks], Wv[:, ks]], axis=1)
        # [C, 768] -> [sub, p, ko, m] (sub-major, 4KB contiguous per (sub, p))
        wqkv_r = wqkv_c.reshape(KT, P, NSUB, P).transpose(2, 1, 0, 3)
        bqkv_c = np.stack(
            [np.concatenate(bqc[2 * i : 2 * i + 2]) for i in range(4)]
            + [bk[ks], bv[ks]],
            axis=1,
        )
        in_maps.append(
            {
                "xt": xts[b],
                "wqkv": np.ascontiguousarray(wqkv_r).astype(np.float16),
                "wo": np.ascontiguousarray(np.concatenate(wocols, axis=0)).astype(
                    np.float16
                ),
                "bqkv": np.ascontiguousarray(bqkv_c),
                "mask": mask,
                "ident": ident,
            }
        )
    return in_maps


def kernel(x, Wq, bq, Wk, bk, Wv, bv, Wo, bo, _trace=False):
    # NTFF tracing is unavailable through this axon client; make sure a
    # stray BASS_TRACE=1 in the environment cannot divert the run path.
    if not _trace:
        os.environ["BASS_NEVER_TRACE"] = "1"
    if "nc" not in _CACHE:
        _CACHE["nc"] = _build()
    nc = _CACHE["nc"]
    in_maps = _prep_inputs(x, Wq, bq, Wk, bk, Wv, bv, Wo, bo)
    res = bass_utils.run_bass_kernel_spmd(
        nc, in_maps, core_ids=list(range(NCORES)), trace=_trace
    )
    bo = np.asarray(bo, dtype=np.float32)
    y = np.zeros((B, T, C), dtype=np.float32)
    for c in range(NCORES):
        y[c // 4] += res.results[c]["y"].astype(np.float32)
    y += bo
    if _trace:
        return y, res
    return y


# revision 70
# speedup vs baseline: 1.0021x; 1.0008x over previous
"""GQA forward kernel for Trainium2, 8-core tensor-parallel (group-aligned).

Problem: B=2, T=2048, D=2048, 32 Q heads / 8 KV heads, head_dim 64, causal.

Sharding: core c = (batch b = c//4, kv-head pair j = c%4).  Each core owns
kv heads {2j, 2j+1} and their 8 q heads for ONE batch.  Each core reads only
its batch's x^T (fp16) and emits a row-parallel partial of the output
projection (fp16); the host sums 4 partials per batch (+ bo).

All matmuls in fp16 with fp32 PSUM accumulation (tolerance is 2e-2; fp16
keeps rel err ~1e-3 and always hits 1.0 cycles/row on the PE cost model).

Per-core dataflow:
  P1 (proj, 4 chunks of 512 tokens): lhsT = wqkv sub [C/16-slices, 128] fp16,
    rhs = x^T chunk -> 6 sub-blocks of 128: subs 0-3 = Q pairs [g0hi | g1hi],
    sub 4 = K2 = [K_g0 | K_g1]^T, sub 5 = V2^T (transposed to natural via PE
    identity-matmul transposes).  v2 layout [kv, 130] = [V_g0 | 1 | V_g1 | 1].
  P2 attention per (pair i, q-chunk of 512): scores transposed,
    S^T[kv, q] for both heads of the pair in one PSUM tile [128,2,512]
    (head g0hi contracts K2[0:64], g1hi contracts K2[64:128]).
    expS = ACT Exp(S/8) -> fp16 SBUF (all kv tiles of the chunk kept live);
    causal via column-sliced matmuls + one triangle mask multiply on
    diagonal tiles.
    AV is flipped: per [128q, 65] output region, lhsT = expS tile (stationary)
    and rhs = [V_h | ones] streams 65 columns - half the engine time of
    streaming query columns.  Each region is one sequential start->stop
    accumulation in its own PSUM bank (interleaved groups within a bank
    break on real hardware).  den lands in column 64, so normalize is a
    per-partition scalar multiply; a PE transpose restores attn^T.
  P3 out-proj: py[t,e] = sum_ks attn^T[128ks, t] @ wo[128ks, e], psum ->
    fp16 staging (DVE) -> one DMA per 128-token row block, deferred a few
    fillers so its wait never blocks the SP queue head.
  Scheduling: attention chunk qc runs right after proj chunk qc; proj chunk
  qc+1 and out-proj chunk qc-1 are emitted one matmul at a time between
  attention tiles as PE filler while ACT works through the exps.
"""

import os

import numpy as np

import concourse.mybir as mybir
import concourse.tile as tile
from concourse import bacc
from concourse import bass_utils

P = 128
B = 2
T = 2048
C = 2048
HD = 64
QH = 32
KVH = 8
NCORES = 8
TCH = 512   # token chunk for projection phase
QCH = 512   # q chunk for attention phase
KT = C // P  # 16 contraction tiles
NSUB = 6     # 4 Q pairs + K2 + V2
f32 = mybir.dt.float32
fp16 = mybir.dt.float16

_CACHE = {}


def _build():
    nc = bacc.Bacc("TRN2", target_bir_lowering=False, debug=False, num_devices=NCORES)

    xt = nc.dram_tensor("xt", [C, T], fp16, kind="ExternalInput")
    # sub-major, pre-rearranged on host: [sub, p, ko, m] so a per-sub load is
    # one 4KB-descriptor DMA
    wqkv = nc.dram_tensor("wqkv", [NSUB, P, KT, P], fp16, kind="ExternalInput")
    wo = nc.dram_tensor("wo", [4 * P, C], fp16, kind="ExternalInput")
    bqkv = nc.dram_tensor("bqkv", [P, NSUB], f32, kind="ExternalInput")
    maskd = nc.dram_tensor("mask", [P, P], fp16, kind="ExternalInput")
    identd = nc.dram_tensor("ident", [P, P], fp16, kind="ExternalInput")
    y = nc.dram_tensor("y", [T, C], fp16, kind="ExternalOutput")

    wo3 = wo.ap().rearrange("(ko p) m -> p ko m", p=P)
    xb = xt.ap().rearrange("(ko p) t -> p ko t", p=P)

    Exp = mybir.ActivationFunctionType.Exp
    mult = mybir.AluOpType.mult
    add = mybir.AluOpType.add

    with tile.TileContext(nc) as tc:
        with (
            tc.tile_pool(name="const", bufs=1) as cpool,
            tc.tile_pool(name="x", bufs=2) as xpool,
            tc.tile_pool(name="res", bufs=1) as apool,
            tc.tile_pool(name="vt", bufs=2) as vtpool,
            tc.tile_pool(name="expS", bufs=16) as wpool,
            tc.tile_pool(name="den", bufs=3) as dpool,
            tc.tile_pool(name="y", bufs=3) as ypool,
            tc.tile_pool(name="psA", bufs=2, space="PSUM") as psumA,
            tc.tile_pool(name="psB", bufs=2, space="PSUM") as psumB,
            tc.tile_pool(name="psC", bufs=2, space="PSUM") as psumC,
        ):
            # ---- constants / weights (resident) ----
            # startup-critical DMA order: wqkv sub0, x chunk 0 halves (the
            # first 16 proj matmuls need only these), then the rest.
            wqkv_subs = []
            for _s in range(NSUB):
                w_s = cpool.tile([P, KT, P], fp16, tag=f"w{_s}", name="w_s")
                wqkv_subs.append(w_s)

            def xch_alloc(tch):
                # two tiles so matmuls on the first 8 K-slices need not wait
                # for the second half's DMA
                xlo = xpool.tile([P, KT // 2, TCH], fp16, tag="xlo", name="xlo")
                xhi = xpool.tile([P, KT // 2, TCH], fp16, tag="xhi", name="xhi")
                tsl = slice(tch * TCH, (tch + 1) * TCH)
                nc.sync.dma_start(xlo[:], xb[:, 0 : KT // 2, tsl])
                nc.sync.dma_start(xhi[:], xb[:, KT // 2 :, tsl])
                return (xlo, xhi)

            # startup-critical order: sub0 weights, x chunk 0 halves, then
            # remaining subs one DMA each (4KB descriptors)
            nc.sync.dma_start(wqkv_subs[0][:], wqkv.ap()[0])
            xch0 = xch_alloc(0)
            for _s in (1, 2, 3, 5, 4):
                nc.sync.dma_start(wqkv_subs[_s][:], wqkv.ap()[_s])
            bqkv_sb = cpool.tile([P, NSUB], f32)
            nc.sync.dma_start(bqkv_sb[:], bqkv.ap())
            mask_sb = cpool.tile([P, P], fp16)
            nc.sync.dma_start(mask_sb[:], maskd.ap())
            ident_sb = cpool.tile([P, P], fp16)
            nc.sync.dma_start(ident_sb[:], identd.ap())
            wo_sb = cpool.tile([P, 4, C], fp16)

            q_sb = apool.tile([P, 4, T], fp16, tag="q")
            k2_sb = apool.tile([P, T], fp16, tag="k2")
            # v2 cols: [V_g0 (0:64) | ones (64) | V_g1 (65:129) | ones (129)]
            v2_sb = apool.tile([P, KT, 130], fp16, tag="v2")
            attn_sb = apool.tile([P, 4, T], fp16, tag="attn")
            nc.gpsimd.memset(v2_sb[:, :, 64:65], 1.0)
            nc.gpsimd.memset(v2_sb[:, :, 129:130], 1.0)

            # ---- filler queues: closures each emitting ~one PE matmul.
            # proj fillers have a deadline (their attention chunk) and pop
            # first; p3 fillers drain opportunistically.
            fillers_proj = []
            fillers_p3 = []

            def pop_filler(k=1):
                for _ in range(k):
                    if fillers_proj:
                        fillers_proj.pop(0)()
                    elif fillers_p3:
                        fillers_p3.pop(0)()

            def make_p3_fillers(qc):
                """Out-proj for token range [q0, q0+qch): one 128-token
                row-block per ts, 4 col-chunks each a 4-matmul psum
                accumulation + DVE copy; one DMA per row-block."""
                q0, qch = qc
                out = []
                prev_dma = None
                for ts in range(q0 // P, (q0 + qch) // P):
                    state = {}
                    mms = []
                    for ec in range(C // QCH):
                        for ks in range(4):
                            def mm(ts=ts, ec=ec, ks=ks, state=state):
                                if ks == 0 and ec == 0:
                                    state["y"] = ypool.tile(
                                        [P, C], fp16, tag="ysb", name="ysb"
                                    )
                                if ks == 0:
                                    state["py"] = psumC.tile(
                                        [P, QCH], f32, tag="pp", name="py"
                                    )
                                nc.tensor.matmul(
                                    state["py"][:],
                                    attn_sb[:, ks, ts * P : (ts + 1) * P],
                                    wo_sb[:, ks, ec * QCH : (ec + 1) * QCH],
                                    start=(ks == 0),
                                    stop=(ks == 3),
                                    skip_group_check=True,
                                )
                                if ks == 3:
                                    nc.vector.tensor_copy(
                                        state["y"][:, ec * QCH : (ec + 1) * QCH],
                                        state["py"][:],
                                    )
                            mm.pe_ns = 213
                            mms.append(mm)

                    def ydma(ts=ts, state=state):
                        nc.sync.dma_start(
                            y.ap()[ts * P : (ts + 1) * P, :], state["y"][:]
                        )

                    ydma.pe_ns = 0

                    # defer each row-block's output DMA a few fillers past its
                    # last staging copy so its wait is satisfied when the SP
                    # queue reaches it
                    out.extend(mms[:4])
                    if prev_dma is not None:
                        out.append(prev_dma)
                    out.extend(mms[4:])
                    prev_dma = ydma
                out.append(prev_dma)
                return out

            # ---- P1: one projection chunk, as a list of filler closures ----
            def proj_fillers(tch, xch):
                tsl = slice(tch * TCH, (tch + 1) * TCH)
                out = []
                tposes = []
                for sub in (0, 1, 2, 3, 5, 4):
                    state = {}
                    for k in range(KT):
                        def mm(sub=sub, k=k, state=state):
                            if k == 0:
                                state["pp"] = psumC.tile(
                                    [P, TCH], f32, tag="pp", name="pp"
                                )
                            nc.tensor.matmul(
                                state["pp"][:],
                                wqkv_subs[sub][:, k, :],
                                xch[k // (KT // 2)][:, k % (KT // 2), :],
                                start=(k == 0),
                                stop=(k == KT - 1),
                                skip_group_check=True,
                            )
                            if k == KT - 1:
                                bias = bqkv_sb[:, sub : sub + 1].to_broadcast(
                                    (P, TCH)
                                )
                                if sub < 4:
                                    nc.vector.tensor_tensor(
                                        q_sb[:, sub, tsl], state["pp"][:], bias, add
                                    )
                                elif sub == 4:
                                    nc.vector.tensor_tensor(
                                        k2_sb[:, tsl], state["pp"][:], bias, add
                                    )
                                else:
                                    state["vt"] = vtpool.tile(
                                        [P, TCH], fp16, tag="vt", name="vt"
                                    )
                                    nc.vector.tensor_tensor(
                                        state["vt"][:], state["pp"][:], bias, add
                                    )
                        mm.pe_ns = 213
                        out.append(mm)
                    if sub == 5:
                        for ts in range(TCH // P):
                            def tpose(ts=ts, tch=tch, state=state):
                                tidx = tch * (TCH // P) + ts
                                pt = psumC.tile([P, P], fp16, tag="pp", name="pt")
                                nc.tensor.transpose(
                                    pt[:],
                                    state["vt"][:, ts * P : (ts + 1) * P],
                                    ident_sb[:],
                                )
                                nc.vector.tensor_copy(v2_sb[:, tidx, 0:64], pt[:, 0:64])
                                nc.vector.tensor_copy(
                                    v2_sb[:, tidx, 65:129], pt[:, 64:128]
                                )
                            tposes.append(tpose)
                out.extend(tposes)
                return out

            def proj_chunk(tch, xch):
                for fn in proj_fillers(tch, xch):
                    fn()

            # ---- P2: attention for one (pair, q-chunk) ----
            # AV is "flipped": expS tiles are the PE stationary operand and V
            # streams through (65 cols instead of up to 512), halving AV
            # engine time.  Output pq[q, d] has the denominator at column 64
            # of each region, so normalize is a per-partition scalar multiply;
            # a PE transpose restores attn^T for the output projection.
            def attn_pair(q0, qch, pair):
                nq = qch // P
                nfull = q0 // P
                ntiles = nfull + nq
                exps = []
                for i in range(ntiles):
                    lo = 0 if i < nfull else (i - nfull) * P
                    nsl = slice(lo, qch)
                    qsl = slice(q0 + lo, q0 + qch)
                    ksl = slice(i * P, (i + 1) * P)
                    ps_s = psumA.tile([P, 2, QCH], f32, tag="ps", name="ps_s")
                    nc.tensor.matmul(
                        ps_s[:, 0, nsl],
                        k2_sb[0:64, ksl],
                        q_sb[0:64, pair, qsl],
                        start=True,
                        stop=True,
                        skip_group_check=True,
                    )
                    nc.tensor.matmul(
                        ps_s[:, 1, nsl],
                        k2_sb[64:128, ksl],
                        q_sb[64:128, pair, qsl],
                        start=True,
                        stop=True,
                        skip_group_check=True,
                    )
                    expS = wpool.tile([P, 2, QCH], fp16, tag="expS")
                    nc.scalar.activation(expS[:, :, nsl], ps_s[:, :, nsl], Exp, scale=0.125)
                    if i >= nfull:
                        j = i - nfull
                        jsl = slice(j * P, (j + 1) * P)
                        nc.vector.tensor_tensor(
                            expS[:, :, jsl],
                            expS[:, :, jsl],
                            mask_sb[:, None, :].to_broadcast((P, 2, P)),
                            mult,
                        )
                    exps.append(expS)
                    pop_filler(2)
                # AV per [128q, 65] region: one sequential accumulation group
                # per PSUM bank; den lands in column 64, so normalize is a
                # per-partition scalar multiply; a PE transpose restores
                # attn^T for the output projection.
                qsl = slice(q0, q0 + qch)
                tp = psumC.tile([P, 4, P], fp16, tag="pp", name="tp")
                for h in range(2):
                    rec = dpool.tile([P, 4, 1], f32, tag="rec")
                    attn_n = dpool.tile([P, 4, HD], fp16, tag="attn_n")
                    for jj in range(nq):
                        pq = psumB.tile([P, 65], f32, tag="pq", name="pq")
                        last = nfull + jj
                        for i in range(last + 1):
                            nc.tensor.matmul(
                                pq[:],
                                exps[i][:, h, jj * P : (jj + 1) * P],
                                v2_sb[:, i, h * 65 : h * 65 + 65],
                                start=(i == 0),
                                stop=(i == last),
                                skip_group_check=True,
                            )
                        nc.vector.reciprocal(rec[:, jj, :], pq[:, 64:65])
                        nc.vector.tensor_tensor(
                            attn_n[:, jj, :],
                            pq[:, 0:64],
                            rec[:, jj, :].to_broadcast((P, HD)),
                            mult,
                        )
                        nc.tensor.transpose(
                            tp[h * HD : (h + 1) * HD, jj, :],
                            attn_n[:, jj, :],
                            ident_sb[:],
                        )
                        pop_filler(1)
                nc.vector.tensor_copy(attn_sb[:, pair, qsl], tp[:, 0:nq, :])

            # ---- main schedule ----
            # proj(0) runs as a block; proj(qc+1) and out-proj(qc-1) pop as
            # fillers between attention tiles of chunk qc, keeping the PE fed
            # while ACT works through the exps.  The last token chunk's
            # attention is split in two so most of its out-proj still has
            # attention to overlap with.
            proj_chunk(0, xch0)
            nc.sync.dma_start(wo_sb[:], wo3)
            sched = [(0, TCH), (TCH, TCH), (2 * TCH, TCH), (3 * TCH, TCH)]
            for s, (q0, qch) in enumerate(sched):
                if s < 3:
                    fillers_proj.extend(proj_fillers(s + 1, xch_alloc(s + 1)))
                for pair in range(4):
                    attn_pair(q0, qch, pair)
                    pop_filler(2)
                # the next chunk's attention needs its projections complete
                while fillers_proj:
                    pop_filler(1)
                fillers_p3.extend(make_p3_fillers((q0, qch)))
            while fillers_p3:
                pop_filler(1)

    nc.compile()
    return nc


def _prep_inputs(x, Wq, bq, Wk, bk, Wv, bv, Wo, bo):
    x = np.ascontiguousarray(np.asarray(x, dtype=np.float32))
    Wq = np.asarray(Wq, dtype=np.float32)
    Wk = np.asarray(Wk, dtype=np.float32)
    Wv = np.asarray(Wv, dtype=np.float32)
    Wo = np.asarray(Wo, dtype=np.float32)
    bq = np.asarray(bq, dtype=np.float32)
    bk = np.asarray(bk, dtype=np.float32)
    bv = np.asarray(bv, dtype=np.float32)

    xts = [np.ascontiguousarray(x[b].T).astype(np.float16) for b in range(B)]
    # mask[kj, qi] = 1 iff kj <= qi  (upper triangular incl. diag)
    mask = np.triu(np.ones((P, P), dtype=np.float16)).copy()
    ident = np.eye(P, dtype=np.float16)
    in_maps = []
    for c in range(NCORES):
        b, j = divmod(c, 4)
        # q heads: g0 = 8j..8j+3 (kv head 2j), g1 = 8j+4..8j+7 (kv 2j+1)
        qcols, wocols, bqc = [], [], []
        for i in range(4):
            for h in (8 * j + i, 8 * j + 4 + i):
                qcols.append(Wq[:, h * HD : (h + 1) * HD])
                wocols.append(Wo[h * HD : (h + 1) * HD, :])
                bqc.append(bq[h * HD : (h + 1) * HD])
        ks = slice(2 * j * HD, (2 * j + 2) * HD)
        wqkv_c = np.concatenate(qcols + [Wk[:, ks], Wv[:, ks]], axis=1)
        # [C, 768] -> [sub, p, ko, m] (sub-major, 4KB contiguous per (sub, p))
        wqkv_r = wqkv_c.reshape(KT, P, NSUB, P).transpose(2, 1, 0, 3)
        bqkv_c = np.stack(
            [np.concatenate(bqc[2 * i : 2 * i + 2]) for i in range(4)]
            + [bk[ks], bv[ks]],
            axis=1,
        )
        in_maps.append(
            {
                "xt": xts[b],
                "wqkv": np.ascontiguousarray(wqkv_r).astype(np.float16),
                "wo": np.ascontiguousarray(np.concatenate(wocols, axis=0)).astype(
                    np.float16
                ),
                "bqkv": np.ascontiguousarray(bqkv_c),
                "mask": mask,
                "ident": ident,
            }
        )
    return in_maps


def kernel(x, Wq, bq, Wk, bk, Wv, bv, Wo, bo, _trace=False):
    # NTFF tracing is unavailable through this axon client; make sure a
    # stray BASS_TRACE=1 in the environment cannot divert the run path.
    if not _trace:
        os.environ["BASS_NEVER_TRACE"] = "1"
    if "nc" not in _CACHE:
        _CACHE["nc"] = _build()
    nc = _CACHE["nc"]
    in_maps = _prep_inputs(x, Wq, bq, Wk, bk, Wv, bv, Wo, bo)
    res = bass_utils.run_bass_kernel_spmd(
        nc, in_maps, core_ids=list(range(NCORES)), trace=_trace
    )
    bo = np.asarray(bo, dtype=np.float32)
    y = np.zeros((B, T, C), dtype=np.float32)
    for c in range(NCORES):
        y[c // 4] += res.results[c]["y"].astype(np.float32)
    y += bo
    if _trace:
        return y, res
    return y
